# revision 66
# baseline (speedup 1.0000x reference)
"""Distributed Trainium2 (Bass/Tile) kernel for nn_Attention_10771777978397.

Strategy (tensor-parallel over heads, 8 NeuronCores):
  - Each core computes Q/K/V projections for its 2 heads (of 16) over the full
    batch, applies RoPE, runs causal attention in a transposed-softmax layout,
    and produces attnT [o_local=256, r=4096] (bf16).
  - One AllToAll per head redistributes attnT so core j holds ALL heads for its
    row slice r_j (512 rows); each core then does the full wo projection for
    its rows and writes its [512, 2048] f32 output slice. Host concatenates.

Q/K are projected WEIGHT-STATIONARY (lhsT = wq/wk column tile, rhs = xT
granule), producing [hd, r] tiles directly in the layout attention needs —
no on-device transposes. RoPE pair-mixing runs as a DVE stream_shuffle
(lane swap l<->l+16 within each 32-lane quadrant): weight rows are permuted
host-side so a pair's even/odd components sit 16 lanes apart, and the sin
table carries the sign flip. V stays x-stationary (natural [r, od] layout
for the PV matmul).

All matmuls run in bf16 with f32 PSUM accumulation. Softmax uses exp without
max-subtraction (logits bounded by construction; masked logits never
computed; the causal boundary is a multiplicative lower-triangular mask on
the diagonal 128x128 block). Softmax denominators in the projection phase
are computed off the PE (DVE k-merge + gpsimd partition_all_reduce, hidden
under dense matmuls); the h=1 block keeps the classic PE ones-matmul since
a 3.6us gpsimd reduce there would delay the sends that trigger A2A1.

The wo projection runs in two passes (h=0 contraction after AllToAll-0,
h=1 after AllToAll-1). All h=1 attention is deferred until after the A2A0
trigger so its ~30us of PE/ACT work hides the first collective's transit;
pass A then hides A2A1's transit. The wo weights prefetch in two halves
(one early on the scalar HWDGE queue, one behind the A2A0 trigger) so the
8.4MB load doesn't contend with the collective's DRAM traffic. Output is
stored bf16 and upcast on host.
"""

import math
import os

import numpy as np
import ml_dtypes

import concourse.bass as bass
import concourse.tile as tile
from concourse.tile import add_dep_helper
from concourse import bacc, mybir
from concourse.bass_isa import ReduceOp
from concourse.bass_utils import run_bass_kernel_spmd

# problem shape (hardcoded per harness contract)
B, S, D, H = 4, 1024, 2048, 16
HD = D // H          # 128
NCORES = 8
HPC = H // NCORES    # 2 heads per core
OL = HPC * HD        # 256 local o-dim
R = B * S            # 4096 rows
RPC = R // NCORES    # 512 output rows per core
NT = R // 128        # 32 r-tiles
ST = S // 128        # 8 s-tiles per batch
NG = R // 512        # 8 granules of 512 rows
SCALE = 1.0 / math.sqrt(HD)

BF16 = mybir.dt.bfloat16
F32 = mybir.dt.float32
I32 = mybir.dt.int32
NPBF16 = ml_dtypes.bfloat16
Copy = mybir.ActivationFunctionType.Copy

# lane swap l <-> l+16 within each 32-lane quadrant (RoPE pair mix)
SWAP_MASK = [(l + 16) % 32 for l in range(32)]

_CACHED = {}


def _build():
    nc = bacc.Bacc("TRN2", target_bir_lowering=False, debug=False,
                   num_devices=NCORES, name="attn_tp")

    xt = nc.declare_dram_parameter("xt", [D, R], BF16, isOutput=False)
    wqkt = nc.declare_dram_parameter("wqkt", [D, 2 * OL], BF16, isOutput=False)
    wvt = nc.declare_dram_parameter("wvt", [D, OL], BF16, isOutput=False)
    wot = nc.declare_dram_parameter("wot", [D, D], BF16, isOutput=False)
    ropec = nc.declare_dram_parameter("ropec", [128, 2, 512], BF16, isOutput=False)
    ropes = nc.declare_dram_parameter("ropes", [128, 2, 512], BF16, isOutput=False)
    tri = nc.declare_dram_parameter("tri", [128, 128], BF16, isOutput=False)
    out = nc.declare_dram_parameter("out", [RPC, D], BF16, isOutput=True)

    # DRAM views with the SBUF-tile structure for single batched DMAs
    xt_v = xt.ap().rearrange("(k p) r -> p k r", p=128)          # [128,16,R]
    wqk_v = wqkt.ap().rearrange("(k p) o -> p k o", p=128)
    wv_v = wvt.ap().rearrange("(k p) o -> p k o", p=128)
    wo_v = wot.ap().rearrange("(k p) o -> p k o", p=128)         # [128,16,D]

    import contextlib
    with tile.TileContext(nc) as tc:
        p1ctx = contextlib.ExitStack()
        with (
            tc.tile_pool(name="persist", bufs=1) as persist,
            tc.tile_pool(name="expp", bufs=6) as expp,
            tc.tile_pool(name="attp", bufs=10) as attp,
            tc.tile_pool(name="normp", bufs=4) as normp,
            tc.tile_pool(name="wopE", bufs=2) as wopE,
            tc.tile_pool(name="dram", bufs=1, space="DRAM") as dram,
        ):
            p2ctx = contextlib.ExitStack()
            qkvp = p2ctx.enter_context(tc.tile_pool(name="qkvp", bufs=1))
            attctx = contextlib.ExitStack()
            # front-phase PSUM budget: scps3 + pops2 + csps1 + projps2 = 8
            # banks (measured best; late-csps + projps3 gave a ~10us faster
            # front but a systematically worse tail)
            scps = attctx.enter_context(
                tc.tile_pool(name="scps", bufs=3, space="PSUM"))
            pops = attctx.enter_context(
                tc.tile_pool(name="pops", bufs=2, space="PSUM"))
            csps = attctx.enter_context(
                tc.tile_pool(name="csps", bufs=1, space="PSUM"))
            QT = qkvp.tile([128, HPC, NT, 128], BF16)   # [hd, h, t, r]
            KT = qkvp.tile([128, HPC, NT, 128], BF16)
            Vsb = qkvp.tile([128, NT, OL], BF16)        # [r, t, o]
            p1 = p1ctx.enter_context(tc.tile_pool(name="p1", bufs=1))
            xtp = p1ctx.enter_context(tc.tile_pool(name="xtp", bufs=2))
            ropep = p1ctx.enter_context(tc.tile_pool(name="ropep", bufs=3))
            projps = p1ctx.enter_context(
                tc.tile_pool(name="projps", bufs=2, space="PSUM"))
            # ---- phase-1 SBUF loads. The q/k weights and the first x granule
            # gate the first projection group, so they go first on their
            # queues; wv/rope/tri are needed ~10us later.
            wqk_sb = p1.tile([128, 16, 2 * OL], BF16)   # [.., (qk, h, hd)]
            wv_sb = p1.tile([128, 16, OL], BF16)
            for kc in range(0, 16, 2):
                nc.sync.dma_start(out=wqk_sb[:, kc:kc + 2, :], in_=wqk_v[:, kc:kc + 2, :])
            for kc in range(0, 16, 4):
                nc.sync.dma_start(out=wv_sb[:, kc:kc + 4, :], in_=wv_v[:, kc:kc + 4, :])
            rc_sb = p1.tile([128, 2, 512], BF16)
            rs_sb = p1.tile([128, 2, 512], BF16)
            nc.sync.dma_start(out=rc_sb[:], in_=ropec.ap())
            nc.sync.dma_start(out=rs_sb[:], in_=ropes.ap())
            tri_sb = persist.tile([128, 128], BF16)
            nc.sync.dma_start(out=tri_sb[:], in_=tri.ap())
            ones_sb = persist.tile([128, 1], BF16)
            nc.vector.memset(ones_sb[:], 1.0)

            # PE pre-warm: dependency-free matmuls run during the initial DMA
            # loads and flip the HAM clock gate to 2.4 GHz before real work
            warm_sb = persist.tile([128, 512], BF16, name="warm_sb")
            nc.vector.memset(warm_sb[:], 0.0)
            for w in range(16):
                w_ps = scps.tile([128, 512], F32, tag="sc", name=f"warm{w}")
                nc.tensor.matmul(out=w_ps[:], lhsT=warm_sb[:, :128], rhs=warm_sb[:],
                                 start=True, stop=True)

            send = [dram.tile([NCORES, 128, RPC], BF16, name=f"send{h}", tag=f"send{h}")
                    for h in range(HPC)]
            recv = [dram.tile([NCORES, 128, RPC], BF16, name=f"recv{h}", tag=f"recv{h}")
                    for h in range(HPC)]

            # ================= Phase 1 + 2 interleaved =========================
            def emit_granule(g):
                par = g % 2
                # x loads ride the sync HWDGE queue: the ring holds only ~2
                # outstanding DMAs, and a queued dma_start blocks every later
                # instruction on its issuing engine — the scalar engine must
                # stay free for the exp/copy stream. Exception: the first two
                # granules go on the (still idle) scalar queue so they stream
                # in parallel with the weight loads on sync.
                eng = nc.scalar if g >= 6 else nc.sync
                xg = xtp.tile([128, 16, 512], BF16, tag="xg", name=f"xg{g}")
                for kc in range(0, 16, 4):
                    eng.dma_start(out=xg[:, kc:kc + 4, :],
                                  in_=xt_v[:, kc:kc + 4, g * 512:(g + 1) * 512])
                # q/k weight-stationary: out [hd=128, r=512] per (qk, head).
                # Sequential groups — alternating qk/v matmuls was tried and
                # regressed ~30us: the single background weight buffer can't
                # pipeline LDWEIGHTS from two alternating weight sources.
                for h in range(HPC):
                    for qk, dst in ((0, QT), (1, KT)):
                        c0 = (qk * HPC + h) * HD
                        ps = projps.tile([128, 512], F32, tag="proj",
                                         name=f"proj{g}_{qk}_{h}")
                        for k in range(16):
                            nc.tensor.matmul(out=ps[:],
                                             lhsT=wqk_sb[:, k, c0:c0 + HD],
                                             rhs=xg[:, k, :],
                                             start=(k == 0), stop=(k == 15))
                        raw = ropep.tile([128, 512], BF16, tag="raw",
                                         name=f"raw{g}_{qk}_{h}")
                        nc.scalar.activation(out=raw[:], in_=ps[:], func=Copy)
                        shuf = ropep.tile([128, 512], BF16, tag="shuf",
                                          name=f"shuf{g}_{qk}_{h}")
                        nc.vector.stream_shuffle(shuf[:].bitcast(I32),
                                                 raw[:].bitcast(I32), SWAP_MASK)
                        t1 = ropep.tile([128, 512], BF16, tag="t1",
                                        name=f"t1{g}_{qk}_{h}")
                        nc.vector.tensor_mul(t1[:], raw[:], rc_sb[:, par, :])
                        t2 = ropep.tile([128, 512], BF16, tag="t2",
                                        name=f"t2{g}_{qk}_{h}")
                        nc.vector.tensor_mul(t2[:], shuf[:], rs_sb[:, par, :])
                        dt = dst[:, h, g * 4:(g + 1) * 4, :].rearrange("p a b -> p (a b)")
                        nc.vector.tensor_add(dt, t1[:], t2[:])
                # v x-stationary: out [r=128, o=256] per 128-row chunk
                for u in range(4):
                    t = g * 4 + u
                    v_ps = projps.tile([128, OL], F32, tag="proj", name=f"vps{t}")
                    for k in range(16):
                        nc.tensor.matmul(out=v_ps[:],
                                         lhsT=xg[:, k, u * 128:(u + 1) * 128],
                                         rhs=wv_sb[:, k, :],
                                         start=(k == 0), stop=(k == 15))
                    nc.vector.tensor_copy(Vsb[:, t, :], v_ps[:])

            def emit_attention(b, h, bc_insts=None, send_insts=None, chunks=(0, 1),
                               send_eng=None, pe_cs=False):
                # pe_cs=False: denominator via DVE merge + gpsimd
                # partition_all_reduce (3.6us/chunk on gpsimd, so only used in
                # the projection phase where gpsimd is idle and the chunk tail
                # hides under dense matmuls). pe_cs=True: classic ones-matmul
                # on the PE — used in the h=1 block where the PE is exp-gated
                # anyway and gpsimd serialization would delay the sends that
                # trigger A2A1.
                for c in chunks:                # sq chunks of 512
                    jblk = b * 2 + c
                    o_ps = pops.tile([128, 512], F32, tag="po", name=f"po{b}_{h}_{c}")
                    njt = 4 * c + 4             # sk tiles for this chunk
                    if pe_cs:
                        cs_ps = csps.tile([1, 512], F32, tag="cs", name=f"cs{b}_{h}_{c}")
                    else:
                        esum = normp.tile([128, 512], F32, tag="esum",
                                          name=f"es{b}_{h}_{c}")
                    # PV (and cs) matmuls are emitted 3 score-tiles behind
                    # their exp: the PE queue is in-order, and a PV issued
                    # right after its score matmul reaches the queue head
                    # ~0.5us before the exp it consumes is ready — the stall
                    # blocks every later score matmul and cascades into the
                    # exp stream. Lag 3 (~0.8us of sc work) covers the exp
                    # latency so PV never head-of-line-blocks.
                    def emit_pv(j):
                        col0 = max(0, (j - 4 * c) * 128)
                        st, sp = (j == 0), (j == njt - 1)
                        ex = exs[j]
                        if pe_cs:
                            nc.tensor.matmul(out=cs_ps[:, col0:], lhsT=ones_sb[:],
                                             rhs=ex[:, col0:], start=st, stop=sp)
                        nc.tensor.matmul(out=o_ps[:, col0:],
                                         lhsT=Vsb[:, b * ST + j, h * 128:(h + 1) * 128],
                                         rhs=ex[:, col0:], start=st, stop=sp)

                    exs = {}
                    for j in range(njt):
                        col0 = max(0, (j - 4 * c) * 128)
                        t0 = b * ST + 4 * c
                        s_ps = scps.tile([128, 512], F32, tag="sc", name=f"sc{b}_{h}_{c}_{j}")
                        nc.tensor.matmul(
                            out=s_ps[:, col0:], lhsT=KT[:, h, b * ST + j, :],
                            rhs=QT[:, h, t0 + col0 // 128:t0 + 4, :],
                            start=True, stop=True)
                        ex = expp.tile([128, 512], BF16, tag="ex", name=f"ex{b}_{h}_{c}_{j}")
                        nc.scalar.activation(
                            out=ex[:, col0:], in_=s_ps[:, col0:],
                            func=mybir.ActivationFunctionType.Exp, scale=SCALE)
                        if j - 4 * c >= 0:      # diagonal block: causal mask
                            nc.vector.tensor_mul(
                                ex[:, col0:col0 + 128], ex[:, col0:col0 + 128], tri_sb[:])
                        exs[j] = ex
                        if pe_cs:
                            pass
                        elif j == 0:            # j=0 is always full-width
                            nc.vector.tensor_copy(esum[:], ex[:])
                        else:
                            nc.vector.tensor_add(esum[:, col0:], esum[:, col0:],
                                                 ex[:, col0:])
                        if j >= 3:
                            emit_pv(j - 3)
                    for j in range(max(0, njt - 3), njt):
                        emit_pv(j)
                    att = attp.tile([128, 512], BF16, tag="att", name=f"att{b}_{h}_{c}")
                    if pe_cs:
                        rcp = normp.tile([1, 512], F32, tag="rcp", name=f"rcp{b}_{h}_{c}")
                        nc.vector.reciprocal_approx_fast(out=rcp[:], in_=cs_ps[:])
                        bc = normp.tile([128, 512], F32, tag="bc", name=f"bc{b}_{h}_{c}")
                        nc.gpsimd.partition_broadcast(bc[:], rcp[:])
                        nc.vector.tensor_mul(att[:], o_ps[:], bc[:])
                    else:
                        nc.gpsimd.partition_all_reduce(esum[:], esum[:], 128,
                                                       ReduceOp.add)
                        rbc = normp.tile([128, 512], F32, tag="rbc", name=f"rbc{b}_{h}_{c}")
                        nc.vector.reciprocal_approx_fast(out=rbc[:], in_=esum[:])
                        nc.vector.tensor_mul(att[:], o_ps[:], rbc[:])
                    eng = send_eng if send_eng is not None else nc.sync
                    snd = eng.dma_start(out=send[h][jblk, :, :], in_=att[:])
                    if send_insts is not None:
                        send_insts.append(snd)

            # Granule pairs run in rotated order [b3, b0, b1, b2]; each batch's
            # h=0 attention is emitted right after its own pair, so its
            # exp-gated chains interleave with the NEXT pair's dense projection
            # matmuls. All h=1 attention is deferred until after the A2A0
            # trigger so ~30us of PE+ACT work covers the collective's transit.
            sends0 = []
            sends1 = []
            emit_granule(6)
            emit_granule(7)
            emit_attention(3, 0, send_insts=sends0)
            # tiny warm-up collective, triggered mid-phase-1 from the gpsimd
            # FIFO: pre-pays the ncfw wake-up and cross-core dispatch skew
            # while the PE is saturated, so the real AllToAlls at the tail see
            # an already-synced, warm collective engine
            warm_in = dram.tile([NCORES, 1], mybir.dt.uint8, name="warm_in", tag="warm_in")
            warm_out = dram.tile([NCORES, 1], mybir.dt.uint8, name="warm_out", tag="warm_out")
            nc.gpsimd.collective_compute(
                "AllToAll", mybir.AluOpType.bypass,
                replica_groups=[list(range(NCORES))],
                ins=[warm_in.opt()], outs=[warm_out.opt()])
            for b in range(2):
                emit_granule(2 * b)
                emit_granule(2 * b + 1)
                emit_attention(b, 0, send_insts=sends0)
            emit_granule(4)
            # first half of the wo weights streams in NOW (scalar HWDGE queue,
            # issue-only cost) so only half the 8.4MB prefetch competes with
            # A2A0's DRAM traffic later
            wts = []
            for dc in range(2):
                wt = wopE.tile([128, 16, 512], BF16, tag="wtE", name=f"wt{dc}")
                nc.scalar.dma_start(
                    out=wt[:], in_=wo_v[:, :, dc * 512:(dc + 1) * 512])
                wts.append(wt)
            # batch 2's first chunk only needs granule 4; it rides granule 5
            emit_attention(2, 0, send_insts=sends0, chunks=(0,))
            emit_granule(5)

            p1ctx.close()   # projection pools released

            # att(2,0,c1) trails alone so A2A0 can fire immediately; pe_cs so
            # its send isn't delayed behind a 3.6us gpsimd reduce
            emit_attention(2, 0, send_insts=sends0, chunks=(1,), pe_cs=True)
            cc0 = nc.gpsimd.collective_compute(
                "AllToAll", mybir.AluOpType.bypass,
                replica_groups=[list(range(NCORES))],
                ins=[send[0].opt()], outs=[recv[0].opt()])

            with (
                tc.tile_pool(name="wop", bufs=2) as wop,
                tc.tile_pool(name="fop", bufs=3) as fop,
                tc.tile_pool(name="fap", bufs=16) as fap,
                tc.tile_pool(name="rtp", bufs=1) as rtp,
            ):
                rT0 = rtp.tile([128, 8, RPC], BF16)   # h=0 o-tiles (k=2i)
                rT1 = rtp.tile([128, 8, RPC], BF16)   # h=1 o-tiles (k=2i+1)

                # second half of the wo weight prefetch on the gpsimd SWDGE
                # queue right behind the A2A0 trigger — overlaps the h=1
                # attention block; its SBUF-reuse fence (freed p1 pools) is
                # already satisfied by the granule-5 matmuls
                for dc in range(2, 4):
                    wt = wop.tile([128, 16, 512], BF16, tag="wt", name=f"wt{dc}")
                    nc.gpsimd.dma_start(
                        out=wt[:], in_=wo_v[:, :, dc * 512:(dc + 1) * 512])
                    wts.append(wt)

                # h=1 attention for all batches: PE/ACT/DVE work covering
                # A2A0's transit while the wo weights stream in. Sends stay on
                # the sync queue (unblocked), so the A2A1 trigger fires the
                # moment the last h=1 chunk lands.
                emit_attention(3, 1, send_insts=sends1, pe_cs=True)
                emit_attention(0, 1, send_insts=sends1, pe_cs=True)
                emit_attention(1, 1, send_insts=sends1, pe_cs=True)
                emit_attention(2, 1, send_insts=sends1, pe_cs=True)
                cc1 = nc.gpsimd.collective_compute(
                    "AllToAll", mybir.AluOpType.bypass,
                    replica_groups=[list(range(NCORES))],
                    ins=[send[1].opt()], outs=[recv[1].opt()])

                # rT0 loads ride the SCALAR queue so the h=1 sends (sync) flow
                # freely; explicit edges after every h=1 send pin them behind
                # the whole h=1 block — the scheduler would otherwise hoist
                # them mid-exp-stream and stall the exps on the A2A0 semaphore.
                rd0 = nc.scalar.dma_start(out=rT0[:, 0:2, :],
                                          in_=recv[0][0:2].rearrange("i p r -> p i r"))
                rd0b = nc.scalar.dma_start(out=rT0[:, 2:8, :],
                                           in_=recv[0][2:8].rearrange("i p r -> p i r"))
                for si in sends1:
                    add_dep_helper(rd0.ins, si.ins, sync=False,
                                   reason="rT0 load after all h1 sends")
                add_dep_helper(rd0b.ins, rd0.ins, sync=False, reason="rT0b after rT0a")

                attctx.close()  # attention psum pools released for the wo passes
                mmctx = contextlib.ExitStack()
                mmps = mmctx.enter_context(
                    tc.tile_pool(name="mmps", bufs=8, space="PSUM"))

                # rT1 loads on sync behind the h=1 sends; first two src blocks
                # land first so pass B's opening matmuls aren't gated on the tail
                rd1 = nc.sync.dma_start(out=rT1[:, 0:2, :],
                                        in_=recv[1][0:2].rearrange("i p r -> p i r"))
                rd1b = nc.sync.dma_start(out=rT1[:, 2:8, :],
                                         in_=recv[1][2:8].rearrange("i p r -> p i r"))
                for si in sends1:
                    add_dep_helper(rd1.ins, si.ins, sync=False,
                                   reason="rT1 load after all h1 sends on sync queue")
                add_dep_helper(rd1b.ins, rd1.ins, sync=False, reason="rT1b after rT1a")

                # wo contraction: per rt the stationary rT block stays loaded
                # across the 4 dc-column groups (accumulating in 4 psum banks),
                # so LDWEIGHTS amortizes 4x and the PE streams at rhs rate.
                # pass A: h=0 half off rT0
                fas = {}
                for rt in range(4):
                    fa_ps = [mmps.tile([128, 512], F32, tag="mm", name=f"faps{rt}_{d}")
                             for d in range(4)]
                    for i in range(8):
                        for dc in range(4):
                            nc.tensor.matmul(out=fa_ps[dc][:],
                                             lhsT=rT0[:, i, rt * 128:(rt + 1) * 128],
                                             rhs=wts[dc][:, 2 * i, :],
                                             start=(i == 0), stop=(i == 7))
                    for dc in range(4):
                        fa = fap.tile([128, 512], BF16, tag="fa", name=f"fa{dc}_{rt}")
                        nc.vector.tensor_copy(fa[:], fa_ps[dc][:])
                        fas[(dc, rt)] = fa

                # pass B: h=1 half + combine + store
                for rt in range(4):
                    f_ps = [mmps.tile([128, 512], F32, tag="mm", name=f"fps{rt}_{d}")
                            for d in range(4)]
                    for i in range(8):
                        for dc in range(4):
                            nc.tensor.matmul(out=f_ps[dc][:],
                                             lhsT=rT1[:, i, rt * 128:(rt + 1) * 128],
                                             rhs=wts[dc][:, 2 * i + 1, :],
                                             start=(i == 0), stop=(i == 7))
                    for dc in range(4):
                        fo = fop.tile([128, 512], BF16, tag="fo")
                        nc.vector.tensor_add(fo[:], f_ps[dc][:], fas[(dc, rt)][:])
                        nc.sync.dma_start(
                            out=out.ap()[rt * 128:(rt + 1) * 128, dc * 512:(dc + 1) * 512], in_=fo[:])

                mmctx.close()

            p2ctx.close()   # QT/KT/Vsb released after the wo pools

    nc.compile()
    return nc


def _prep_inputs(x, freqs, wq, wk, wv, wo):
    x = np.asarray(x, np.float32)
    freqs = np.asarray(freqs, np.float32)
    wq = np.asarray(wq, np.float32)
    wk = np.asarray(wk, np.float32)
    wv = np.asarray(wv, np.float32)
    wo = np.asarray(wo, np.float32)

    xt = np.ascontiguousarray(x.reshape(R, D).T).astype(NPBF16)
    wot = np.ascontiguousarray(wo.T).astype(NPBF16)

    # quadrant-pair layout: partition p = 32q + l holds pair j = 16q + (l%16),
    # even component for l<16, odd for l>=16 — so the stream_shuffle lane swap
    # (l <-> l+16) exchanges a pair's components
    lperm = np.empty(128, np.int64)
    jidx = np.empty(128, np.int64)
    sgn = np.empty(128, np.float32)
    for p in range(128):
        q, l = divmod(p, 32)
        j = 16 * q + (l % 16)
        jidx[p] = j
        lperm[p] = 2 * j + (0 if l < 16 else 1)
        sgn[p] = -1.0 if l < 16 else 1.0

    cosf = np.cos(freqs)   # [S, 64]
    sinf = np.sin(freqs)
    ropec = np.ascontiguousarray(cosf[:, jidx].T).reshape(128, 2, 512).astype(NPBF16)
    ropes = np.ascontiguousarray(
        (sinf[:, jidx].T * sgn[:, None])).reshape(128, 2, 512).astype(NPBF16)

    tri = np.tril(np.ones((128, 128), np.float32)).T.copy()  # tri[p,f]=1 if p<=f
    tri = tri.astype(NPBF16)

    in_maps = []
    for core in range(NCORES):
        qkcols = []
        vcols = []
        for hh in range(HPC):
            head = core * HPC + hh
            qkcols.append(head * HD + lperm)
            vcols.append(np.arange(head * HD, (head + 1) * HD))
        vcols = np.concatenate(vcols)
        # column order: (qk, h, hd)
        wqk_host = np.concatenate(
            [wq[qkcols[0], :].T, wq[qkcols[1], :].T,
             wk[qkcols[0], :].T, wk[qkcols[1], :].T], axis=1)
        in_maps.append({
            "xt": xt,
            "wqkt": np.ascontiguousarray(wqk_host).astype(NPBF16),
            "wvt": np.ascontiguousarray(wv[vcols, :].T).astype(NPBF16),
            "wot": wot,
            "ropec": ropec,
            "ropes": ropes,
            "tri": tri,
        })
    return in_maps


def kernel(x, freqs, mask, wq, wk, wv, wo, start_pos, _trace=False):
    # mask is the standard causal mask (applied structurally on-device);
    # start_pos is 0 for this problem shape.
    if "nc" not in _CACHED:
        _CACHED["nc"] = _build()
    nc = _CACHED["nc"]
    in_maps = _prep_inputs(x, freqs, wq, wk, wv, wo)
    # warmup execution: settles PJRT dispatch, NRT comm init, and core-start
    # skew so the measured execution reflects steady-state kernel time
    if os.environ.get("ATTN_TP_WARMUP", "1") == "1" and "warm" not in _CACHED:
        run_bass_kernel_spmd(nc, in_maps, core_ids=list(range(NCORES)), trace=False)
        _CACHED["warm"] = True
    res = run_bass_kernel_spmd(nc, in_maps, core_ids=list(range(NCORES)), trace=_trace)
    out = np.concatenate([res.results[j]["out"] for j in range(NCORES)], axis=0)
    kernel.last_results = res
    return out.reshape(B, S, D).astype(np.float32)



# revision 67
# speedup vs baseline: 1.0235x; 1.0235x over previous
"""Distributed Trainium2 (Bass/Tile) kernel for nn_Attention_10771777978397.

Strategy (tensor-parallel over heads, 8 NeuronCores):
  - Each core computes Q/K/V projections for its 2 heads (of 16) over the full
    batch, applies RoPE, runs causal attention in a transposed-softmax layout,
    and produces attnT [o_local=256, r=4096] (bf16).
  - One AllToAll per head redistributes attnT so core j holds ALL heads for its
    row slice r_j (512 rows); each core then does the full wo projection for
    its rows and writes its [512, 2048] f32 output slice. Host concatenates.

Q/K are projected WEIGHT-STATIONARY (lhsT = wq/wk column tile, rhs = xT
granule), producing [hd, r] tiles directly in the layout attention needs —
no on-device transposes. RoPE pair-mixing runs as a DVE stream_shuffle
(lane swap l<->l+16 within each 32-lane quadrant): weight rows are permuted
host-side so a pair's even/odd components sit 16 lanes apart, and the sin
table carries the sign flip. V stays x-stationary (natural [r, od] layout
for the PV matmul).

All matmuls run in bf16 with f32 PSUM accumulation. Softmax uses exp without
max-subtraction (logits bounded by construction; masked logits never
computed; the causal boundary is a multiplicative lower-triangular mask on
the diagonal 128x128 block). Softmax denominators in the projection phase
are computed off the PE (DVE k-merge + gpsimd partition_all_reduce, hidden
under dense matmuls); the h=1 block keeps the classic PE ones-matmul since
a 3.6us gpsimd reduce there would delay the sends that trigger A2A1.

The wo projection runs in two passes (h=0 contraction after AllToAll-0,
h=1 after AllToAll-1). All h=1 attention is deferred until after the A2A0
trigger so its ~30us of PE/ACT work hides the first collective's transit;
pass A then hides A2A1's transit. The wo weights prefetch in two halves
(one early on the scalar HWDGE queue, one behind the A2A0 trigger) so the
8.4MB load doesn't contend with the collective's DRAM traffic. Output is
stored bf16 and upcast on host.
"""

import math
import os

import numpy as np
import ml_dtypes

import concourse.bass as bass
import concourse.tile as tile
from concourse.tile import add_dep_helper
from concourse import bacc, mybir
from concourse.bass_isa import ReduceOp
from concourse.bass_utils import run_bass_kernel_spmd

# problem shape (hardcoded per harness contract)
B, S, D, H = 4, 1024, 2048, 16
HD = D // H          # 128
NCORES = 8
HPC = H // NCORES    # 2 heads per core
OL = HPC * HD        # 256 local o-dim
R = B * S            # 4096 rows
RPC = R // NCORES    # 512 output rows per core
NT = R // 128        # 32 r-tiles
ST = S // 128        # 8 s-tiles per batch
NG = R // 512        # 8 granules of 512 rows
SCALE = 1.0 / math.sqrt(HD)

BF16 = mybir.dt.bfloat16
F32 = mybir.dt.float32
I32 = mybir.dt.int32
NPBF16 = ml_dtypes.bfloat16
Copy = mybir.ActivationFunctionType.Copy

# lane swap l <-> l+16 within each 32-lane quadrant (RoPE pair mix)
SWAP_MASK = [(l + 16) % 32 for l in range(32)]

_CACHED = {}


def _build():
    nc = bacc.Bacc("TRN2", target_bir_lowering=False, debug=False,
                   num_devices=NCORES, name="attn_tp")

    xt = nc.declare_dram_parameter("xt", [D, R], BF16, isOutput=False)
    wqkt = nc.declare_dram_parameter("wqkt", [D, 2 * OL], BF16, isOutput=False)
    wvt = nc.declare_dram_parameter("wvt", [D, OL], BF16, isOutput=False)
    wot = nc.declare_dram_parameter("wot", [D, D], BF16, isOutput=False)
    ropec = nc.declare_dram_parameter("ropec", [128, 2, 512], BF16, isOutput=False)
    ropes = nc.declare_dram_parameter("ropes", [128, 2, 512], BF16, isOutput=False)
    tri = nc.declare_dram_parameter("tri", [128, 128], BF16, isOutput=False)
    out = nc.declare_dram_parameter("out", [RPC, D], BF16, isOutput=True)

    # DRAM views with the SBUF-tile structure for single batched DMAs
    xt_v = xt.ap().rearrange("(k p) r -> p k r", p=128)          # [128,16,R]
    wqk_v = wqkt.ap().rearrange("(k p) o -> p k o", p=128)
    wv_v = wvt.ap().rearrange("(k p) o -> p k o", p=128)
    wo_v = wot.ap().rearrange("(k p) o -> p k o", p=128)         # [128,16,D]

    import contextlib
    with tile.TileContext(nc) as tc:
        p1ctx = contextlib.ExitStack()
        with (
            tc.tile_pool(name="persist", bufs=1) as persist,
            tc.tile_pool(name="expp", bufs=6) as expp,
            tc.tile_pool(name="attp", bufs=10) as attp,
            tc.tile_pool(name="normp", bufs=4) as normp,
            tc.tile_pool(name="wopE", bufs=2) as wopE,
            tc.tile_pool(name="dram", bufs=1, space="DRAM") as dram,
        ):
            p2ctx = contextlib.ExitStack()
            qkvp = p2ctx.enter_context(tc.tile_pool(name="qkvp", bufs=1))
            attctx = contextlib.ExitStack()
            # front-phase PSUM budget: scps3 + pops2 + csps1 + projps2 = 8
            # banks (measured best; late-csps + projps3 gave a ~10us faster
            # front but a systematically worse tail)
            scps = attctx.enter_context(
                tc.tile_pool(name="scps", bufs=3, space="PSUM"))
            pops = attctx.enter_context(
                tc.tile_pool(name="pops", bufs=2, space="PSUM"))
            csps = attctx.enter_context(
                tc.tile_pool(name="csps", bufs=1, space="PSUM"))
            QT = qkvp.tile([128, HPC, NT, 128], BF16)   # [hd, h, t, r]
            KT = qkvp.tile([128, HPC, NT, 128], BF16)
            Vsb = qkvp.tile([128, NT, OL], BF16)        # [r, t, o]
            p1 = p1ctx.enter_context(tc.tile_pool(name="p1", bufs=1))
            xtp = p1ctx.enter_context(tc.tile_pool(name="xtp", bufs=2))
            ropep = p1ctx.enter_context(tc.tile_pool(name="ropep", bufs=3))
            projps = p1ctx.enter_context(
                tc.tile_pool(name="projps", bufs=2, space="PSUM"))
            # ---- phase-1 SBUF loads. The q/k weights and the first x granule
            # gate the first projection group, so they go first on their
            # queues; wv/rope/tri are needed ~10us later.
            wqk_sb = p1.tile([128, 16, 2 * OL], BF16)   # [.., (qk, h, hd)]
            wv_sb = p1.tile([128, 16, OL], BF16)
            for kc in range(0, 16, 2):
                nc.sync.dma_start(out=wqk_sb[:, kc:kc + 2, :], in_=wqk_v[:, kc:kc + 2, :])
            for kc in range(0, 16, 4):
                nc.sync.dma_start(out=wv_sb[:, kc:kc + 4, :], in_=wv_v[:, kc:kc + 4, :])
            rc_sb = p1.tile([128, 2, 512], BF16)
            rs_sb = p1.tile([128, 2, 512], BF16)
            nc.sync.dma_start(out=rc_sb[:], in_=ropec.ap())
            nc.sync.dma_start(out=rs_sb[:], in_=ropes.ap())
            tri_sb = persist.tile([128, 128], BF16)
            nc.sync.dma_start(out=tri_sb[:], in_=tri.ap())
            ones_sb = persist.tile([128, 1], BF16)
            nc.vector.memset(ones_sb[:], 1.0)

            # PE pre-warm: dependency-free matmuls run during the initial DMA
            # loads and flip the HAM clock gate to 2.4 GHz before real work
            warm_sb = persist.tile([128, 512], BF16, name="warm_sb")
            nc.vector.memset(warm_sb[:], 0.0)
            for w in range(16):
                w_ps = scps.tile([128, 512], F32, tag="sc", name=f"warm{w}")
                nc.tensor.matmul(out=w_ps[:], lhsT=warm_sb[:, :128], rhs=warm_sb[:],
                                 start=True, stop=True)

            send = [dram.tile([NCORES, 128, RPC], BF16, name=f"send{h}", tag=f"send{h}")
                    for h in range(HPC)]
            recv = [dram.tile([NCORES, 128, RPC], BF16, name=f"recv{h}", tag=f"recv{h}")
                    for h in range(HPC)]

            # ================= Phase 1 + 2 interleaved =========================
            def emit_granule(g):
                par = g % 2
                # x loads ride the sync HWDGE queue: the ring holds only ~2
                # outstanding DMAs, and a queued dma_start blocks every later
                # instruction on its issuing engine — the scalar engine must
                # stay free for the exp/copy stream. Exception: the first two
                # granules go on the (still idle) scalar queue so they stream
                # in parallel with the weight loads on sync.
                eng = nc.scalar if g >= 6 else nc.sync
                xg = xtp.tile([128, 16, 512], BF16, tag="xg", name=f"xg{g}")
                for kc in range(0, 16, 4):
                    eng.dma_start(out=xg[:, kc:kc + 4, :],
                                  in_=xt_v[:, kc:kc + 4, g * 512:(g + 1) * 512])
                # q/k weight-stationary: out [hd=128, r=512] per (qk, head).
                # Sequential groups — alternating qk/v matmuls was tried and
                # regressed ~30us: the single background weight buffer can't
                # pipeline LDWEIGHTS from two alternating weight sources.
                for h in range(HPC):
                    for qk, dst in ((0, QT), (1, KT)):
                        c0 = (qk * HPC + h) * HD
                        ps = projps.tile([128, 512], F32, tag="proj",
                                         name=f"proj{g}_{qk}_{h}")
                        for k in range(16):
                            nc.tensor.matmul(out=ps[:],
                                             lhsT=wqk_sb[:, k, c0:c0 + HD],
                                             rhs=xg[:, k, :],
                                             start=(k == 0), stop=(k == 15))
                        raw = ropep.tile([128, 512], BF16, tag="raw",
                                         name=f"raw{g}_{qk}_{h}")
                        nc.scalar.activation(out=raw[:], in_=ps[:], func=Copy)
                        shuf = ropep.tile([128, 512], BF16, tag="shuf",
                                          name=f"shuf{g}_{qk}_{h}")
                        nc.vector.stream_shuffle(shuf[:].bitcast(I32),
                                                 raw[:].bitcast(I32), SWAP_MASK)
                        t1 = ropep.tile([128, 512], BF16, tag="t1",
                                        name=f"t1{g}_{qk}_{h}")
                        nc.vector.tensor_mul(t1[:], raw[:], rc_sb[:, par, :])
                        t2 = ropep.tile([128, 512], BF16, tag="t2",
                                        name=f"t2{g}_{qk}_{h}")
                        nc.vector.tensor_mul(t2[:], shuf[:], rs_sb[:, par, :])
                        dt = dst[:, h, g * 4:(g + 1) * 4, :].rearrange("p a b -> p (a b)")
                        nc.vector.tensor_add(dt, t1[:], t2[:])
                # v x-stationary: out [r=128, o=256] per 128-row chunk
                for u in range(4):
                    t = g * 4 + u
                    v_ps = projps.tile([128, OL], F32, tag="proj", name=f"vps{t}")
                    for k in range(16):
                        nc.tensor.matmul(out=v_ps[:],
                                         lhsT=xg[:, k, u * 128:(u + 1) * 128],
                                         rhs=wv_sb[:, k, :],
                                         start=(k == 0), stop=(k == 15))
                    nc.vector.tensor_copy(Vsb[:, t, :], v_ps[:])

            def emit_attention(b, h, bc_insts=None, send_insts=None, chunks=(0, 1),
                               send_eng=None, pe_cs=False):
                # pe_cs=False: denominator via DVE merge + gpsimd
                # partition_all_reduce (3.6us/chunk on gpsimd, so only used in
                # the projection phase where gpsimd is idle and the chunk tail
                # hides under dense matmuls). pe_cs=True: classic ones-matmul
                # on the PE — used in the h=1 block where the PE is exp-gated
                # anyway and gpsimd serialization would delay the sends that
                # trigger A2A1.
                for c in chunks:                # sq chunks of 512
                    jblk = b * 2 + c
                    o_ps = pops.tile([128, 512], F32, tag="po", name=f"po{b}_{h}_{c}")
                    njt = 4 * c + 4             # sk tiles for this chunk
                    if pe_cs:
                        cs_ps = csps.tile([1, 512], F32, tag="cs", name=f"cs{b}_{h}_{c}")
                    else:
                        esum = normp.tile([128, 512], F32, tag="esum",
                                          name=f"es{b}_{h}_{c}")
                    for j in range(njt):
                        col0 = max(0, (j - 4 * c) * 128)
                        t0 = b * ST + 4 * c
                        s_ps = scps.tile([128, 512], F32, tag="sc", name=f"sc{b}_{h}_{c}_{j}")
                        nc.tensor.matmul(
                            out=s_ps[:, col0:], lhsT=KT[:, h, b * ST + j, :],
                            rhs=QT[:, h, t0 + col0 // 128:t0 + 4, :],
                            start=True, stop=True)
                        ex = expp.tile([128, 512], BF16, tag="ex", name=f"ex{b}_{h}_{c}_{j}")
                        nc.scalar.activation(
                            out=ex[:, col0:], in_=s_ps[:, col0:],
                            func=mybir.ActivationFunctionType.Exp, scale=SCALE)
                        if j - 4 * c >= 0:      # diagonal block: causal mask
                            nc.vector.tensor_mul(
                                ex[:, col0:col0 + 128], ex[:, col0:col0 + 128], tri_sb[:])
                        st, sp = (j == 0), (j == njt - 1)
                        if pe_cs:
                            nc.tensor.matmul(out=cs_ps[:, col0:], lhsT=ones_sb[:],
                                             rhs=ex[:, col0:], start=st, stop=sp)
                        elif st:                # j=0 is always full-width
                            nc.vector.tensor_copy(esum[:], ex[:])
                        else:
                            nc.vector.tensor_add(esum[:, col0:], esum[:, col0:],
                                                 ex[:, col0:])
                        nc.tensor.matmul(out=o_ps[:, col0:],
                                         lhsT=Vsb[:, b * ST + j, h * 128:(h + 1) * 128],
                                         rhs=ex[:, col0:], start=st, stop=sp)
                    att = attp.tile([128, 512], BF16, tag="att", name=f"att{b}_{h}_{c}")
                    if pe_cs:
                        rcp = normp.tile([1, 512], F32, tag="rcp", name=f"rcp{b}_{h}_{c}")
                        nc.vector.reciprocal_approx_fast(out=rcp[:], in_=cs_ps[:])
                        bc = normp.tile([128, 512], F32, tag="bc", name=f"bc{b}_{h}_{c}")
                        nc.gpsimd.partition_broadcast(bc[:], rcp[:])
                        nc.vector.tensor_mul(att[:], o_ps[:], bc[:])
                    else:
                        nc.gpsimd.partition_all_reduce(esum[:], esum[:], 128,
                                                       ReduceOp.add)
                        rbc = normp.tile([128, 512], F32, tag="rbc", name=f"rbc{b}_{h}_{c}")
                        nc.vector.reciprocal_approx_fast(out=rbc[:], in_=esum[:])
                        nc.vector.tensor_mul(att[:], o_ps[:], rbc[:])
                    eng = send_eng if send_eng is not None else nc.sync
                    snd = eng.dma_start(out=send[h][jblk, :, :], in_=att[:])
                    if send_insts is not None:
                        send_insts.append(snd)

            # Granule pairs run in rotated order [b3, b0, b1, b2]; each batch's
            # h=0 attention is emitted right after its own pair, so its
            # exp-gated chains interleave with the NEXT pair's dense projection
            # matmuls. All h=1 attention is deferred until after the A2A0
            # trigger so ~30us of PE+ACT work covers the collective's transit.
            sends0 = []
            sends1 = []
            emit_granule(6)
            emit_granule(7)
            emit_attention(3, 0, send_insts=sends0)
            # tiny warm-up collective, triggered mid-phase-1 from the gpsimd
            # FIFO: pre-pays the ncfw wake-up and cross-core dispatch skew
            # while the PE is saturated, so the real AllToAlls at the tail see
            # an already-synced, warm collective engine
            warm_in = dram.tile([NCORES, 1], mybir.dt.uint8, name="warm_in", tag="warm_in")
            warm_out = dram.tile([NCORES, 1], mybir.dt.uint8, name="warm_out", tag="warm_out")
            nc.gpsimd.collective_compute(
                "AllToAll", mybir.AluOpType.bypass,
                replica_groups=[list(range(NCORES))],
                ins=[warm_in.opt()], outs=[warm_out.opt()])
            for b in range(2):
                emit_granule(2 * b)
                emit_granule(2 * b + 1)
                emit_attention(b, 0, send_insts=sends0)
            emit_granule(4)
            # first half of the wo weights streams in NOW (scalar HWDGE queue,
            # issue-only cost) so only half the 8.4MB prefetch competes with
            # A2A0's DRAM traffic later
            wts = []
            for dc in range(2):
                wt = wopE.tile([128, 16, 512], BF16, tag="wtE", name=f"wt{dc}")
                nc.scalar.dma_start(
                    out=wt[:], in_=wo_v[:, :, dc * 512:(dc + 1) * 512])
                wts.append(wt)
            # batch 2's first chunk only needs granule 4; it rides granule 5
            emit_attention(2, 0, send_insts=sends0, chunks=(0,))
            emit_granule(5)

            p1ctx.close()   # projection pools released

            # att(2,0,c1) trails alone so A2A0 can fire immediately; pe_cs so
            # its send isn't delayed behind a 3.6us gpsimd reduce
            emit_attention(2, 0, send_insts=sends0, chunks=(1,), pe_cs=True)
            cc0 = nc.gpsimd.collective_compute(
                "AllToAll", mybir.AluOpType.bypass,
                replica_groups=[list(range(NCORES))],
                ins=[send[0].opt()], outs=[recv[0].opt()])

            with (
                tc.tile_pool(name="wop", bufs=2) as wop,
                tc.tile_pool(name="fop", bufs=3) as fop,
                tc.tile_pool(name="fap", bufs=16) as fap,
                tc.tile_pool(name="rtp", bufs=1) as rtp,
            ):
                rT0 = rtp.tile([128, 8, RPC], BF16)   # h=0 o-tiles (k=2i)
                rT1 = rtp.tile([128, 8, RPC], BF16)   # h=1 o-tiles (k=2i+1)

                # second half of the wo weight prefetch on the gpsimd SWDGE
                # queue right behind the A2A0 trigger — overlaps the h=1
                # attention block; its SBUF-reuse fence (freed p1 pools) is
                # already satisfied by the granule-5 matmuls
                for dc in range(2, 4):
                    wt = wop.tile([128, 16, 512], BF16, tag="wt", name=f"wt{dc}")
                    nc.gpsimd.dma_start(
                        out=wt[:], in_=wo_v[:, :, dc * 512:(dc + 1) * 512])
                    wts.append(wt)

                # h=1 attention for all batches: PE/ACT/DVE work covering
                # A2A0's transit while the wo weights stream in. Sends stay on
                # the sync queue (unblocked), so the A2A1 trigger fires the
                # moment the last h=1 chunk lands.
                emit_attention(3, 1, send_insts=sends1, pe_cs=True)
                emit_attention(0, 1, send_insts=sends1, pe_cs=True)
                emit_attention(1, 1, send_insts=sends1, pe_cs=True)
                emit_attention(2, 1, send_insts=sends1, pe_cs=True)
                cc1 = nc.gpsimd.collective_compute(
                    "AllToAll", mybir.AluOpType.bypass,
                    replica_groups=[list(range(NCORES))],
                    ins=[send[1].opt()], outs=[recv[1].opt()])

                # rT0 loads ride the SCALAR queue so the h=1 sends (sync) flow
                # freely; explicit edges after every h=1 send pin them behind
                # the whole h=1 block — the scheduler would otherwise hoist
                # them mid-exp-stream and stall the exps on the A2A0 semaphore.
                rd0 = nc.scalar.dma_start(out=rT0[:, 0:2, :],
                                          in_=recv[0][0:2].rearrange("i p r -> p i r"))
                rd0b = nc.scalar.dma_start(out=rT0[:, 2:8, :],
                                           in_=recv[0][2:8].rearrange("i p r -> p i r"))
                for si in sends1:
                    add_dep_helper(rd0.ins, si.ins, sync=False,
                                   reason="rT0 load after all h1 sends")
                add_dep_helper(rd0b.ins, rd0.ins, sync=False, reason="rT0b after rT0a")

                attctx.close()  # attention psum pools released for the wo passes
                mmctx = contextlib.ExitStack()
                mmps = mmctx.enter_context(
                    tc.tile_pool(name="mmps", bufs=8, space="PSUM"))

                # rT1 loads on sync behind the h=1 sends; first two src blocks
                # land first so pass B's opening matmuls aren't gated on the tail
                rd1 = nc.sync.dma_start(out=rT1[:, 0:2, :],
                                        in_=recv[1][0:2].rearrange("i p r -> p i r"))
                rd1b = nc.sync.dma_start(out=rT1[:, 2:8, :],
                                         in_=recv[1][2:8].rearrange("i p r -> p i r"))
                for si in sends1:
                    add_dep_helper(rd1.ins, si.ins, sync=False,
                                   reason="rT1 load after all h1 sends on sync queue")
                add_dep_helper(rd1b.ins, rd1.ins, sync=False, reason="rT1b after rT1a")

                # wo contraction: per rt the stationary rT block stays loaded
                # across the 4 dc-column groups (accumulating in 4 psum banks),
                # so LDWEIGHTS amortizes 4x and the PE streams at rhs rate.
                # pass A: h=0 half off rT0
                fas = {}
                for rt in range(4):
                    fa_ps = [mmps.tile([128, 512], F32, tag="mm", name=f"faps{rt}_{d}")
                             for d in range(4)]
                    for i in range(8):
                        for dc in range(4):
                            nc.tensor.matmul(out=fa_ps[dc][:],
                                             lhsT=rT0[:, i, rt * 128:(rt + 1) * 128],
                                             rhs=wts[dc][:, 2 * i, :],
                                             start=(i == 0), stop=(i == 7))
                    for dc in range(4):
                        fa = fap.tile([128, 512], BF16, tag="fa", name=f"fa{dc}_{rt}")
                        nc.vector.tensor_copy(fa[:], fa_ps[dc][:])
                        fas[(dc, rt)] = fa

                # pass B: h=1 half + combine + store
                for rt in range(4):
                    f_ps = [mmps.tile([128, 512], F32, tag="mm", name=f"fps{rt}_{d}")
                            for d in range(4)]
                    for i in range(8):
                        for dc in range(4):
                            nc.tensor.matmul(out=f_ps[dc][:],
                                             lhsT=rT1[:, i, rt * 128:(rt + 1) * 128],
                                             rhs=wts[dc][:, 2 * i + 1, :],
                                             start=(i == 0), stop=(i == 7))
                    for dc in range(4):
                        fo = fop.tile([128, 512], BF16, tag="fo")
                        nc.vector.tensor_add(fo[:], f_ps[dc][:], fas[(dc, rt)][:])
                        nc.sync.dma_start(
                            out=out.ap()[rt * 128:(rt + 1) * 128, dc * 512:(dc + 1) * 512], in_=fo[:])

                mmctx.close()

            p2ctx.close()   # QT/KT/Vsb released after the wo pools

    nc.compile()
    return nc


def _prep_inputs(x, freqs, wq, wk, wv, wo):
    x = np.asarray(x, np.float32)
    freqs = np.asarray(freqs, np.float32)
    wq = np.asarray(wq, np.float32)
    wk = np.asarray(wk, np.float32)
    wv = np.asarray(wv, np.float32)
    wo = np.asarray(wo, np.float32)

    xt = np.ascontiguousarray(x.reshape(R, D).T).astype(NPBF16)
    wot = np.ascontiguousarray(wo.T).astype(NPBF16)

    # quadrant-pair layout: partition p = 32q + l holds pair j = 16q + (l%16),
    # even component for l<16, odd for l>=16 — so the stream_shuffle lane swap
    # (l <-> l+16) exchanges a pair's components
    lperm = np.empty(128, np.int64)
    jidx = np.empty(128, np.int64)
    sgn = np.empty(128, np.float32)
    for p in range(128):
        q, l = divmod(p, 32)
        j = 16 * q + (l % 16)
        jidx[p] = j
        lperm[p] = 2 * j + (0 if l < 16 else 1)
        sgn[p] = -1.0 if l < 16 else 1.0

    cosf = np.cos(freqs)   # [S, 64]
    sinf = np.sin(freqs)
    ropec = np.ascontiguousarray(cosf[:, jidx].T).reshape(128, 2, 512).astype(NPBF16)
    ropes = np.ascontiguousarray(
        (sinf[:, jidx].T * sgn[:, None])).reshape(128, 2, 512).astype(NPBF16)

    tri = np.tril(np.ones((128, 128), np.float32)).T.copy()  # tri[p,f]=1 if p<=f
    tri = tri.astype(NPBF16)

    in_maps = []
    for core in range(NCORES):
        qkcols = []
        vcols = []
        for hh in range(HPC):
            head = core * HPC + hh
            qkcols.append(head * HD + lperm)
            vcols.append(np.arange(head * HD, (head + 1) * HD))
        vcols = np.concatenate(vcols)
        # column order: (qk, h, hd)
        wqk_host = np.concatenate(
            [wq[qkcols[0], :].T, wq[qkcols[1], :].T,
             wk[qkcols[0], :].T, wk[qkcols[1], :].T], axis=1)
        in_maps.append({
            "xt": xt,
            "wqkt": np.ascontiguousarray(wqk_host).astype(NPBF16),
            "wvt": np.ascontiguousarray(wv[vcols, :].T).astype(NPBF16),
            "wot": wot,
            "ropec": ropec,
            "ropes": ropes,
            "tri": tri,
        })
    return in_maps


def kernel(x, freqs, mask, wq, wk, wv, wo, start_pos, _trace=False):
    # mask is the standard causal mask (applied structurally on-device);
    # start_pos is 0 for this problem shape.
    if "nc" not in _CACHED:
        _CACHED["nc"] = _build()
    nc = _CACHED["nc"]
    in_maps = _prep_inputs(x, freqs, wq, wk, wv, wo)
    # warmup execution: settles PJRT dispatch, NRT comm init, and core-start
    # skew so the measured execution reflects steady-state kernel time
    if os.environ.get("ATTN_TP_WARMUP", "1") == "1" and "warm" not in _CACHED:
        run_bass_kernel_spmd(nc, in_maps, core_ids=list(range(NCORES)), trace=False)
        _CACHED["warm"] = True
    res = run_bass_kernel_spmd(nc, in_maps, core_ids=list(range(NCORES)), trace=_trace)
    out = np.concatenate([res.results[j]["out"] for j in range(NCORES)], axis=0)
    kernel.last_results = res
    return out.reshape(B, S, D).astype(np.float32)

